# revision 26
# baseline (speedup 1.0000x reference)
"""Trainium2 Bass kernel for nn_BrickVectorEdgeModel (GNN edge MLP).

Computes, for each batch b and node pair (i, j):
    f   = relu(W_b @ relu(W_a @ bv + b_a + W_xy @ xy + b_xy) + b_b)   per node
    e1  = relu(W1 @ f[j] + W2 @ f[i] + b_ca)                          per edge
    e2  = relu(W_cb @ e1 + b_cb)
    e3  = relu(W_cc @ e2 + b_cc)
    out = W_out @ e3 + b_out                                          (2 channels)

Sharding: the (B=4, N=192) x N edge grid has 768 i-rows; each of the 8
cores takes 96 consecutive rows, which always fall inside a single batch
b = core//2.  Host permutes that batch's 192 nodes so the core's own 96
i-rows come first; every core then runs the identical program (SPMD) on
its own node set.

The per-node front of the model (f, then u = W1@f for all 192 j-columns
and vpb = W2@f + b_ca for the core's 96 i-rows) is 0.3% of the FLOPs but
would gate the edge phase behind 2.6 MB of weight DMA; it runs on the
host in fp32 and u/vpb ship as inputs.  The device runs the per-edge
99.7%: e1 pointwise, the two 512x512 GEMMs, and the 2-channel output
layer, all in fp16 (fp32 PSUM accumulate).

The output layer is col-tiled: the four 128-row contraction k-tiles run
as four concurrent matmuls in distinct 32-column PE groups
(tile_position=(0, 32g)), each covering a distinct 128-column quarter of
the chunk, so the layer costs ~1 PE cycle/edge instead of 4.  It is also
software-pipelined one chunk behind (issued between the next chunk's e2
and e3 matmul groups) so it never stalls on e3's bias ops.  Partial
[2 x 128] results land on partition pairs 32g..32g+1 and are copied
(+bias) to a staging tile that batches 4 chunks per output DMA.
Dummy matmuls on a zeroed tile warm the PE clock gate (HAM) while the
input DMAs land.
"""

import numpy as np

import concourse.bass as bass
import concourse.mybir as mybir
import concourse.tile as tile
from concourse import bacc
from concourse.bass_utils import run_bass_kernel_spmd

P = 128
H = 512          # hidden width
B = 4
N = 192          # nodes per batch
NCORES = 8
RLOC = 96        # edge-grid rows per core
EDGES = RLOC * N             # flat edge columns per core (18432)
CHUNK = 512
NCHUNK = EDGES // CHUNK      # 36
SGRP = 4                     # chunks per output-DMA supergroup
NSG = NCHUNK // SGRP         # 9

# weight blob layout (fp16): name -> (offset_cols, size_cols), [128 x WCOLS]
_layout = [
    ("wcb", 4 * H),
    ("wcc", 4 * H),
    ("wout", 4 * 2),   # W_out^T as 4 k-tiles of [128, 2]
]
OFF = {}
_c = 0
for _n, _s in _layout:
    OFF[_n] = (_c, _s)
    _c += _s
WCOLS = _c

# bias blob layout (fp32): [128 x BCOLS]
_blayout = [("bcb", 4), ("bcc", 4), ("bout", 1)]
BOFF = {}
_c = 0
for _n, _s in _blayout:
    BOFF[_n] = (_c, _s)
    _c += _s
BCOLS = _c


def _to_tiles(w):
    """[K, M] (K = 4*128 contraction) -> [128, 4, M] stationary layout."""
    K, M = w.shape
    return w.reshape(K // P, P, M).transpose(1, 0, 2)


def _pack_weights(W_cb, b_cb, W_cc, b_cc, W_out, b_out):
    blob = np.zeros((P, WCOLS), np.float16)

    def put(name, arr3):  # arr3: [128, n_k, M]
        off, sz = OFF[name]
        blob[:, off:off + sz] = arr3.reshape(P, -1).astype(np.float16)

    put("wcb", _to_tiles(W_cb.T.astype(np.float32)))
    put("wcc", _to_tiles(W_cc.T.astype(np.float32)))
    put("wout", _to_tiles(W_out.T.astype(np.float32)))        # [128, 4, 2]

    bblob = np.zeros((P, BCOLS), np.float32)

    def putb(name, vec):  # [512] -> [128, 4]
        off, sz = BOFF[name]
        bblob[:, off:off + sz] = vec.astype(np.float32).reshape(4, P).T

    putb("bcb", b_cb)
    putb("bcc", b_cc)
    off, _ = BOFF["bout"]
    for g in range(4):  # partition 32g+ch holds b_out[ch] (col-tiled out layer)
        bblob[32 * g:32 * g + 2, off] = np.asarray(b_out, np.float32)
    return blob, bblob


def _node_phase(brick_vectors, xy, W_xy, b_xy, W_a, b_a, W_b, b_b, W_ca, b_ca):
    """Host fp32 per-node MLP -> (u[B,N,512], vpb[B,N,512])."""
    bv = np.asarray(brick_vectors, np.float32)
    xyf = np.asarray(xy, np.float32)
    f = bv @ np.asarray(W_a, np.float32).T + np.asarray(b_a, np.float32) \
        + xyf @ np.asarray(W_xy, np.float32).T + np.asarray(b_xy, np.float32)
    f = np.maximum(f, 0.0)
    f = np.maximum(f @ np.asarray(W_b, np.float32).T
                   + np.asarray(b_b, np.float32), 0.0)
    W_ca = np.asarray(W_ca, np.float32)
    u = f @ W_ca[:, :H].T                                     # [B, N, 512]
    vpb = f @ W_ca[:, H:].T + np.asarray(b_ca, np.float32)    # [B, N, 512]
    return u, vpb


def _pack_u(u_b, perm):
    """[N, 512] fp32 -> [128, 4, N] fp16 k-tile layout."""
    return u_b[perm].T.reshape(4, P, N).transpose(1, 0, 2).astype(np.float16)


def _pack_v(vpb_b, perm):
    """[N, 512] fp32 -> [128, 4, RLOC] fp16 k-tile layout (own rows only)."""
    return vpb_b[perm[:RLOC]].T.reshape(4, P, RLOC).transpose(1, 0, 2).astype(
        np.float16)


def _build():
    f32 = mybir.dt.float32
    Relu = mybir.ActivationFunctionType.Relu
    Ident = mybir.ActivationFunctionType.Identity
    add = mybir.AluOpType.add
    amax = mybir.AluOpType.max

    f16 = mybir.dt.float16
    nc = bacc.Bacc(None, target_bir_lowering=False)
    wblob = nc.declare_dram_parameter("wblob", [P, WCOLS], f16, isOutput=False)
    bblob = nc.declare_dram_parameter("bblob", [P, BCOLS], f32, isOutput=False)
    ublob = nc.declare_dram_parameter("ublob", [P, 4, N], f16, isOutput=False)
    vblob = nc.declare_dram_parameter("vblob", [P, 4, RLOC], f16, isOutput=False)
    y = nc.declare_dram_parameter("y", [4, 2, NSG, SGRP, P], f32, isOutput=True)

    with tile.TileContext(nc) as tc:
        with tc.tile_pool(name="wf", bufs=1) as wf, \
             tc.tile_pool(name="stp", bufs=1) as stp, \
             tc.tile_pool(name="ep", bufs=2) as ep, \
             tc.tile_pool(name="outp", bufs=2) as outp, \
             tc.tile_pool(name="psE", bufs=7, space="PSUM") as psE, \
             tc.tile_pool(name="psO", bufs=1, space="PSUM") as psO:

            bias_t = wf.tile([P, BCOLS], f32, tag="bias")

            def bias(name, m):
                off, _ = BOFF[name]
                return bias_t[:, off + m:off + m + 1]

            # input DMAs split across the Sync and ACT DGE queues, ordered
            # by when the data is needed: u/vpb gate e1, wcb gates the first
            # e2 matmul, wcc/wout trail by one matmul group.
            # u/vpb first (gate e1), then the weight stages column-split at
            # the wcb/wcc boundary and partition-halved across the two DGE
            # queues (keeps per-partition segments at 4KB for DMA rate).
            # warm-up first in program order: the DVE memset has no input
            # deps, so the dummy matmuls start right after the framework
            # preamble and keep the PE busy (HAM at full rate) while the
            # input DMAs land.
            warm = wf.tile([P, CHUNK], f16, tag="warm")
            nc.vector.memset(warm[:], 0.0)
            pw = psO.tile([P, CHUNK], f32, tag="psO", name="pw")
            for _ in range(15):
                nc.tensor.matmul(pw[:], warm[:, :P], warm[:],
                                 start=True, stop=True)

            CUTW = OFF["wcc"][0]
            u = wf.tile([P, 4, N], f16, tag="u")
            v16 = wf.tile([P, 4, RLOC], f16, tag="v16")
            vpb = wf.tile([P, 4, RLOC], f32, tag="vpb")
            stW = stp.tile([P, WCOLS], f16, tag="stW")
            nc.sync.dma_start(u[:], ublob[:])
            nc.sync.dma_start(v16[:], vblob[:])
            nc.sync.dma_start(bias_t[:], bblob[:])
            nc.scalar.dma_start(stW[:, :CUTW], wblob[:, :CUTW])
            nc.gpsimd.dma_start(stW[:, CUTW:], wblob[:, CUTW:])
            nc.gpsimd.tensor_copy(vpb[:], v16[:])

            def wslice(name, nk, m):
                off, sz = OFF[name]
                assert sz == nk * m
                return stW[:, off:off + sz].rearrange("p (a b) -> p a b", b=m)

            wcb = wslice("wcb", 4, H)
            wcc = wslice("wcc", 4, H)
            wout = wslice("wout", 4, 2)

            # ---- edge phase: 36 chunks of 512 edge columns.  The 2-channel
            #      out layer is pipelined one chunk behind: chunk cc-1's out
            #      quads are issued between chunk cc's e2 and e3 matmul
            #      groups, so they never stall on e3's pointwise ops. ----
            state = {"ob": None, "po": None, "e3": None}

            def out_quads(e3t):
                # 4 concurrent col-group matmuls, k-chain per group; group g
                # covers chunk columns [128g, 128g+128)
                po = psO.tile([P, CHUNK], f32, tag="psO", name="po")
                for k in range(4):
                    for g in range(4):
                        nc.tensor.matmul(
                            po[32 * g:32 * g + 2, 128 * g:128 * g + 128],
                            wout[:, k, :], e3t[:, k, 128 * g:128 * g + 128],
                            start=(k == 0), stop=(k == 3),
                            tile_position=(0, 32 * g))
                return po

            def out_drain(ccx):
                po = state["po"]
                last = ccx == NCHUNK - 1
                if ccx % SGRP == 0:
                    state["ob"] = outp.tile([P, SGRP * CHUNK], f32, tag="ob",
                                            name="ob")
                ob = state["ob"]
                slot = (ccx % SGRP) * CHUNK
                for g in range(4):
                    src = po[32 * g:32 * g + 2, 128 * g:128 * g + 128]
                    dst = ob[32 * g:32 * g + 2,
                             slot + 128 * g:slot + 128 * g + 128]
                    bo = bias("bout", 0)[32 * g:32 * g + 2]
                    # final chunk: all copies on DVE (faster) so the ACT
                    # engine is free to trigger its share of the last DMAs
                    if last or g % 2 == 0:
                        nc.vector.tensor_scalar_add(dst, src, bo)
                    else:
                        nc.scalar.activation(dst, src, Ident, bias=bo, scale=1.0)
                if ccx % SGRP == SGRP - 1:
                    s = ccx // SGRP
                    obv = ob.rearrange("p (c w) -> p c w", w=CHUNK)
                    for g in range(4):
                        eng = nc.scalar if (last and g % 2) else nc.sync
                        eng.dma_start(
                            y[g, :, s, :, :],
                            obv[32 * g:32 * g + 2, :, 128 * g:128 * g + 128])

            for cc in range(NCHUNK):
                e1 = ep.tile([P, 4, CHUNK], f16, tag="e1")
                f0 = cc * CHUNK
                r_lo = f0 // N
                r_hi = (f0 + CHUNK - 1) // N
                for kt in range(4):
                    for rl in range(r_lo, r_hi + 1):
                        cs = max(f0, rl * N)
                        ce = min(f0 + CHUNK, (rl + 1) * N)
                        if kt >= 2:
                            nc.vector.tensor_scalar(
                                e1[:, kt, cs - f0:ce - f0],
                                u[:, kt, cs - rl * N:ce - rl * N],
                                vpb[:, kt, rl:rl + 1], 0.0, add, amax)
                        else:
                            nc.scalar.activation(
                                e1[:, kt, cs - f0:ce - f0],
                                u[:, kt, cs - rl * N:ce - rl * N],
                                Relu, bias=vpb[:, kt, rl:rl + 1], scale=1.0)

                e2 = ep.tile([P, 4, CHUNK], f16, tag="e2")
                for m in range(4):
                    pt = psE.tile([P, CHUNK], f32, tag="psE")
                    for k in range(4):
                        nc.tensor.matmul(pt[:], wcb[:, k, m * P:(m + 1) * P],
                                         e1[:, k, :], start=(k == 0), stop=(k == 3))
                    nc.vector.tensor_scalar(e2[:, m, :], pt[:],
                                            bias("bcb", m), 0.0, add, amax)

                if state["e3"] is not None:
                    state["po"] = out_quads(state["e3"])

                e3 = ep.tile([P, 4, CHUNK], f16, tag="e3")
                for m in range(4):
                    pt = psE.tile([P, CHUNK], f32, tag="psE")
                    for k in range(4):
                        nc.tensor.matmul(pt[:], wcc[:, k, m * P:(m + 1) * P],
                                         e2[:, k, :], start=(k == 0), stop=(k == 3))
                    # final chunk: alternate engines so m=3's bias (which
                    # gates the last out quads) isn't queued behind m=0..2
                    if cc == NCHUNK - 1 and m % 2 == 0:
                        nc.scalar.activation(e3[:, m, :], pt[:], Relu,
                                             bias=bias("bcc", m), scale=1.0)
                    else:
                        nc.vector.tensor_scalar(e3[:, m, :], pt[:],
                                                bias("bcc", m), 0.0, add, amax)

                if state["po"] is not None:
                    out_drain(cc - 1)
                    state["po"] = None
                state["e3"] = e3

            state["po"] = out_quads(state["e3"])
            out_drain(NCHUNK - 1)

    nc.compile()
    return nc


_cache = {}


def _get_nc():
    if "nc" not in _cache:
        _cache["nc"] = _build()
    return _cache["nc"]


def _make_in_maps(brick_vectors, xy, W_xy, b_xy, W_a, b_a, W_b, b_b,
                  W_ca, b_ca, W_cb, b_cb, W_cc, b_cc, W_out, b_out):
    blob, bblob = _pack_weights(W_cb, b_cb, W_cc, b_cc, W_out, b_out)
    u, vpb = _node_phase(brick_vectors, xy, W_xy, b_xy, W_a, b_a,
                         W_b, b_b, W_ca, b_ca)
    perms = []
    in_maps = []
    for c in range(NCORES):
        b, half = c // 2, c % 2
        perm = np.concatenate([np.arange(96) + 96 * half,
                               np.arange(96) + 96 * (1 - half)])
        perms.append((b, perm))
        in_maps.append({
            "wblob": blob,
            "bblob": bblob,
            "ublob": _pack_u(u[b], perm),
            "vblob": _pack_v(vpb[b], perm),
        })
    return in_maps, perms


def kernel(brick_vectors, xy, W_xy, b_xy, W_a, b_a, W_b, b_b,
           W_ca, b_ca, W_cb, b_cb, W_cc, b_cc, W_out, b_out):
    in_maps, perms = _make_in_maps(
        brick_vectors, xy, W_xy, b_xy, W_a, b_a, W_b, b_b,
        W_ca, b_ca, W_cb, b_cb, W_cc, b_cc, W_out, b_out)

    nc = _get_nc()
    res = run_bass_kernel_spmd(nc, in_maps, list(range(NCORES)))

    out = np.empty((B, N, N, 2), np.float32)
    for c in range(NCORES):
        b, perm = perms[c]
        yc = res.results[c]["y"]                  # [4, 2, 9, 4, 128]
        # edge col = (s*4 + c4)*512 + 128*g + col
        flat = yc.transpose(1, 2, 3, 0, 4).reshape(2, EDGES)
        yc2 = flat.reshape(2, RLOC, N)            # [2, rl, jj]
        out[b][np.ix_(perm[:RLOC], perm)] = yc2.transpose(1, 2, 0)
    return out


# revision 27
# speedup vs baseline: 1.0117x; 1.0117x over previous
"""Trainium2 Bass kernel for nn_BrickVectorEdgeModel (GNN edge MLP).

Computes, for each batch b and node pair (i, j):
    f   = relu(W_b @ relu(W_a @ bv + b_a + W_xy @ xy + b_xy) + b_b)   per node
    e1  = relu(W1 @ f[j] + W2 @ f[i] + b_ca)                          per edge
    e2  = relu(W_cb @ e1 + b_cb)
    e3  = relu(W_cc @ e2 + b_cc)
    out = W_out @ e3 + b_out                                          (2 channels)

Sharding: the (B=4, N=192) x N edge grid has 768 i-rows; each of the 8
cores takes 96 consecutive rows, which always fall inside a single batch
b = core//2.  Host permutes that batch's 192 nodes so the core's own 96
i-rows come first; every core then runs the identical program (SPMD) on
its own node set.

The per-node front of the model (f, then u = W1@f for all 192 j-columns
and vpb = W2@f + b_ca for the core's 96 i-rows) is 0.3% of the FLOPs but
would gate the edge phase behind 2.6 MB of weight DMA; it runs on the
host in fp32 and u/vpb ship as inputs.  The device runs the per-edge
99.7%: e1 pointwise, the two 512x512 GEMMs, and the 2-channel output
layer, all in fp16 (fp32 PSUM accumulate).

The output layer is col-tiled: the four 128-row contraction k-tiles run
as four concurrent matmuls in distinct 32-column PE groups
(tile_position=(0, 32g)), each covering a distinct 128-column quarter of
the chunk, so the layer costs ~1 PE cycle/edge instead of 4.  It is also
software-pipelined one chunk behind (issued between the next chunk's e2
and e3 matmul groups) so it never stalls on e3's bias ops.  Partial
[2 x 128] results land on partition pairs 32g..32g+1 and are copied
(+bias) to a staging tile that batches 4 chunks per output DMA.
Dummy matmuls on a zeroed tile warm the PE clock gate (HAM) while the
input DMAs land.
"""

import numpy as np

import concourse.bass as bass
import concourse.mybir as mybir
import concourse.tile as tile
from concourse import bacc
from concourse.bass_utils import run_bass_kernel_spmd

P = 128
H = 512          # hidden width
B = 4
N = 192          # nodes per batch
NCORES = 8
RLOC = 96        # edge-grid rows per core
EDGES = RLOC * N             # flat edge columns per core (18432)
CHUNK = 512
NCHUNK = EDGES // CHUNK      # 36
SGRP = 4                     # chunks per output-DMA supergroup
NSG = NCHUNK // SGRP         # 9

# weight blob layout (fp16): name -> (offset_cols, size_cols), [128 x WCOLS]
_layout = [
    ("wcb", 4 * H),
    ("wcc", 4 * H),
    ("wout", 4 * 2),   # W_out^T as 4 k-tiles of [128, 2]
]
OFF = {}
_c = 0
for _n, _s in _layout:
    OFF[_n] = (_c, _s)
    _c += _s
WCOLS = _c

# bias blob layout (fp32): [128 x BCOLS]
_blayout = [("bcb", 4), ("bcc", 4), ("bout", 1)]
BOFF = {}
_c = 0
for _n, _s in _blayout:
    BOFF[_n] = (_c, _s)
    _c += _s
BCOLS = _c


def _to_tiles(w):
    """[K, M] (K = 4*128 contraction) -> [128, 4, M] stationary layout."""
    K, M = w.shape
    return w.reshape(K // P, P, M).transpose(1, 0, 2)


def _pack_weights(W_cb, b_cb, W_cc, b_cc, W_out, b_out):
    blob = np.zeros((P, WCOLS), np.float16)

    def put(name, arr3):  # arr3: [128, n_k, M]
        off, sz = OFF[name]
        blob[:, off:off + sz] = arr3.reshape(P, -1).astype(np.float16)

    put("wcb", _to_tiles(W_cb.T.astype(np.float32)))
    put("wcc", _to_tiles(W_cc.T.astype(np.float32)))
    put("wout", _to_tiles(W_out.T.astype(np.float32)))        # [128, 4, 2]

    bblob = np.zeros((P, BCOLS), np.float32)

    def putb(name, vec):  # [512] -> [128, 4]
        off, sz = BOFF[name]
        bblob[:, off:off + sz] = vec.astype(np.float32).reshape(4, P).T

    putb("bcb", b_cb)
    putb("bcc", b_cc)
    off, _ = BOFF["bout"]
    for g in range(4):  # partition 32g+ch holds b_out[ch] (col-tiled out layer)
        bblob[32 * g:32 * g + 2, off] = np.asarray(b_out, np.float32)
    return blob, bblob


def _node_phase(brick_vectors, xy, W_xy, b_xy, W_a, b_a, W_b, b_b, W_ca, b_ca):
    """Host fp32 per-node MLP -> (u[B,N,512], vpb[B,N,512])."""
    bv = np.asarray(brick_vectors, np.float32)
    xyf = np.asarray(xy, np.float32)
    f = bv @ np.asarray(W_a, np.float32).T + np.asarray(b_a, np.float32) \
        + xyf @ np.asarray(W_xy, np.float32).T + np.asarray(b_xy, np.float32)
    f = np.maximum(f, 0.0)
    f = np.maximum(f @ np.asarray(W_b, np.float32).T
                   + np.asarray(b_b, np.float32), 0.0)
    W_ca = np.asarray(W_ca, np.float32)
    u = f @ W_ca[:, :H].T                                     # [B, N, 512]
    vpb = f @ W_ca[:, H:].T + np.asarray(b_ca, np.float32)    # [B, N, 512]
    return u, vpb


def _pack_u(u_b, perm):
    """[N, 512] fp32 -> [128, 4, N] fp16 k-tile layout."""
    return u_b[perm].T.reshape(4, P, N).transpose(1, 0, 2).astype(np.float16)


def _pack_v(vpb_b, perm):
    """[N, 512] fp32 -> [128, 4, RLOC] fp32 k-tile layout (own rows only)."""
    return np.ascontiguousarray(
        vpb_b[perm[:RLOC]].T.reshape(4, P, RLOC).transpose(1, 0, 2))


def _build():
    f32 = mybir.dt.float32
    Relu = mybir.ActivationFunctionType.Relu
    Ident = mybir.ActivationFunctionType.Identity
    add = mybir.AluOpType.add
    amax = mybir.AluOpType.max

    f16 = mybir.dt.float16
    nc = bacc.Bacc(None, target_bir_lowering=False)
    wblob = nc.declare_dram_parameter("wblob", [P, WCOLS], f16, isOutput=False)
    bblob = nc.declare_dram_parameter("bblob", [P, BCOLS], f32, isOutput=False)
    ublob = nc.declare_dram_parameter("ublob", [P, 4, N], f16, isOutput=False)
    vblob = nc.declare_dram_parameter("vblob", [P, 4, RLOC], f32, isOutput=False)
    y = nc.declare_dram_parameter("y", [4, 2, NSG, SGRP, P], f32, isOutput=True)

    with tile.TileContext(nc) as tc:
        with tc.tile_pool(name="wf", bufs=1) as wf, \
             tc.tile_pool(name="stp", bufs=1) as stp, \
             tc.tile_pool(name="ep", bufs=2) as ep, \
             tc.tile_pool(name="outp", bufs=2) as outp, \
             tc.tile_pool(name="psE", bufs=7, space="PSUM") as psE, \
             tc.tile_pool(name="psO", bufs=1, space="PSUM") as psO:

            bias_t = wf.tile([P, BCOLS], f32, tag="bias")

            def bias(name, m):
                off, _ = BOFF[name]
                return bias_t[:, off + m:off + m + 1]

            # input DMAs split across the Sync and ACT DGE queues, ordered
            # by when the data is needed: u/vpb gate e1, wcb gates the first
            # e2 matmul, wcc/wout trail by one matmul group.
            # u/vpb first (gate e1), then the weight stages column-split at
            # the wcb/wcc boundary and partition-halved across the two DGE
            # queues (keeps per-partition segments at 4KB for DMA rate).
            CUTW = OFF["wcc"][0]
            HW2 = CUTW // 2
            u = wf.tile([P, 4, N], f16, tag="u")
            vpb = wf.tile([P, 4, RLOC], f32, tag="vpb")
            stW = stp.tile([P, WCOLS], f16, tag="stW")
            nc.sync.dma_start(u[:], ublob[:])
            nc.scalar.dma_start(bias_t[:], bblob[:])
            nc.scalar.dma_start(vpb[:], vblob[:])
            nc.sync.dma_start(stW[:, :HW2], wblob[:, :HW2])
            nc.scalar.dma_start(stW[:, HW2:CUTW], wblob[:, HW2:CUTW])
            nc.sync.dma_start(stW[:, CUTW:CUTW + HW2], wblob[:, CUTW:CUTW + HW2])
            nc.scalar.dma_start(stW[:, CUTW + HW2:], wblob[:, CUTW + HW2:])

            # warm-up: dummy matmuls on a zeroed tile keep the PE busy while
            # the input DMAs land, so the HAM clock gate is at full rate
            # (2.4 GHz) by the time real work starts.
            warm = wf.tile([P, CHUNK], f16, tag="warm")
            nc.vector.memset(warm[:], 0.0)
            pw = psO.tile([P, CHUNK], f32, tag="psO", name="pw")
            for _ in range(20):
                nc.tensor.matmul(pw[:], warm[:, :P], warm[:],
                                 start=True, stop=True)

            def wslice(name, nk, m):
                off, sz = OFF[name]
                assert sz == nk * m
                return stW[:, off:off + sz].rearrange("p (a b) -> p a b", b=m)

            wcb = wslice("wcb", 4, H)
            wcc = wslice("wcc", 4, H)
            wout = wslice("wout", 4, 2)

            # ---- edge phase: 36 chunks of 512 edge columns.  The 2-channel
            #      out layer is pipelined one chunk behind: chunk cc-1's out
            #      quads are issued between chunk cc's e2 and e3 matmul
            #      groups, so they never stall on e3's pointwise ops. ----
            state = {"ob": None, "po": None, "e3": None}

            def out_quads(e3t):
                # 4 concurrent col-group matmuls, k-chain per group; group g
                # covers chunk columns [128g, 128g+128)
                po = psO.tile([P, CHUNK], f32, tag="psO", name="po")
                for k in range(4):
                    for g in range(4):
                        nc.tensor.matmul(
                            po[32 * g:32 * g + 2, 128 * g:128 * g + 128],
                            wout[:, k, :], e3t[:, k, 128 * g:128 * g + 128],
                            start=(k == 0), stop=(k == 3),
                            tile_position=(0, 32 * g))
                return po

            def out_drain(ccx):
                po = state["po"]
                last = ccx == NCHUNK - 1
                if ccx % SGRP == 0:
                    state["ob"] = outp.tile([P, SGRP * CHUNK], f32, tag="ob",
                                            name="ob")
                ob = state["ob"]
                slot = (ccx % SGRP) * CHUNK
                for g in range(4):
                    src = po[32 * g:32 * g + 2, 128 * g:128 * g + 128]
                    dst = ob[32 * g:32 * g + 2,
                             slot + 128 * g:slot + 128 * g + 128]
                    bo = bias("bout", 0)[32 * g:32 * g + 2]
                    # final chunk: all copies on DVE (faster) so the ACT
                    # engine is free to trigger its share of the last DMAs
                    if last or g % 2 == 0:
                        nc.vector.tensor_scalar_add(dst, src, bo)
                    else:
                        nc.scalar.activation(dst, src, Ident, bias=bo, scale=1.0)
                if ccx % SGRP == SGRP - 1:
                    s = ccx // SGRP
                    obv = ob.rearrange("p (c w) -> p c w", w=CHUNK)
                    for g in range(4):
                        eng = nc.scalar if (last and g % 2) else nc.sync
                        eng.dma_start(
                            y[g, :, s, :, :],
                            obv[32 * g:32 * g + 2, :, 128 * g:128 * g + 128])

            for cc in range(NCHUNK):
                e1 = ep.tile([P, 4, CHUNK], f16, tag="e1")
                f0 = cc * CHUNK
                r_lo = f0 // N
                r_hi = (f0 + CHUNK - 1) // N
                for kt in range(4):
                    for rl in range(r_lo, r_hi + 1):
                        cs = max(f0, rl * N)
                        ce = min(f0 + CHUNK, (rl + 1) * N)
                        if kt >= 2:
                            nc.vector.tensor_scalar(
                                e1[:, kt, cs - f0:ce - f0],
                                u[:, kt, cs - rl * N:ce - rl * N],
                                vpb[:, kt, rl:rl + 1], 0.0, add, amax)
                        else:
                            nc.scalar.activation(
                                e1[:, kt, cs - f0:ce - f0],
                                u[:, kt, cs - rl * N:ce - rl * N],
                                Relu, bias=vpb[:, kt, rl:rl + 1], scale=1.0)

                e2 = ep.tile([P, 4, CHUNK], f16, tag="e2")
                for m in range(4):
                    pt = psE.tile([P, CHUNK], f32, tag="psE")
                    for k in range(4):
                        nc.tensor.matmul(pt[:], wcb[:, k, m * P:(m + 1) * P],
                                         e1[:, k, :], start=(k == 0), stop=(k == 3))
                    nc.vector.tensor_scalar(e2[:, m, :], pt[:],
                                            bias("bcb", m), 0.0, add, amax)

                if state["e3"] is not None:
                    state["po"] = out_quads(state["e3"])

                e3 = ep.tile([P, 4, CHUNK], f16, tag="e3")
                for m in range(4):
                    pt = psE.tile([P, CHUNK], f32, tag="psE")
                    for k in range(4):
                        nc.tensor.matmul(pt[:], wcc[:, k, m * P:(m + 1) * P],
                                         e2[:, k, :], start=(k == 0), stop=(k == 3))
                    # final chunk: alternate engines so m=3's bias (which
                    # gates the last out quads) isn't queued behind m=0..2
                    if cc == NCHUNK - 1 and m % 2 == 0:
                        nc.scalar.activation(e3[:, m, :], pt[:], Relu,
                                             bias=bias("bcc", m), scale=1.0)
                    else:
                        nc.vector.tensor_scalar(e3[:, m, :], pt[:],
                                                bias("bcc", m), 0.0, add, amax)

                if state["po"] is not None:
                    out_drain(cc - 1)
                    state["po"] = None
                state["e3"] = e3

            state["po"] = out_quads(state["e3"])
            out_drain(NCHUNK - 1)

    nc.compile()
    return nc


_cache = {}


def _get_nc():
    if "nc" not in _cache:
        _cache["nc"] = _build()
    return _cache["nc"]


def _make_in_maps(brick_vectors, xy, W_xy, b_xy, W_a, b_a, W_b, b_b,
                  W_ca, b_ca, W_cb, b_cb, W_cc, b_cc, W_out, b_out):
    blob, bblob = _pack_weights(W_cb, b_cb, W_cc, b_cc, W_out, b_out)
    u, vpb = _node_phase(brick_vectors, xy, W_xy, b_xy, W_a, b_a,
                         W_b, b_b, W_ca, b_ca)
    perms = []
    in_maps = []
    for c in range(NCORES):
        b, half = c // 2, c % 2
        perm = np.concatenate([np.arange(96) + 96 * half,
                               np.arange(96) + 96 * (1 - half)])
        perms.append((b, perm))
        in_maps.append({
            "wblob": blob,
            "bblob": bblob,
            "ublob": _pack_u(u[b], perm),
            "vblob": _pack_v(vpb[b], perm),
        })
    return in_maps, perms


def kernel(brick_vectors, xy, W_xy, b_xy, W_a, b_a, W_b, b_b,
           W_ca, b_ca, W_cb, b_cb, W_cc, b_cc, W_out, b_out):
    in_maps, perms = _make_in_maps(
        brick_vectors, xy, W_xy, b_xy, W_a, b_a, W_b, b_b,
        W_ca, b_ca, W_cb, b_cb, W_cc, b_cc, W_out, b_out)

    nc = _get_nc()
    res = run_bass_kernel_spmd(nc, in_maps, list(range(NCORES)))

    out = np.empty((B, N, N, 2), np.float32)
    for c in range(NCORES):
        b, perm = perms[c]
        yc = res.results[c]["y"]                  # [4, 2, 9, 4, 128]
        # edge col = (s*4 + c4)*512 + 128*g + col
        flat = yc.transpose(1, 2, 3, 0, 4).reshape(2, EDGES)
        yc2 = flat.reshape(2, RLOC, N)            # [2, rl, jj]
        out[b][np.ix_(perm[:RLOC], perm)] = yc2.transpose(1, 2, 0)
    return out
